# revision 13
# baseline (speedup 1.0000x reference)

# AudioMamba (bidirectional Mamba encoder) on 8 TRN2 NeuronCores.
# Sharding: data-parallel over batch (1 sample per core), no collectives.
# Layout: feature-major [channels-on-partitions, time-on-free], W=524 cols:
#   cols [0:4) left pad (zeros for causal conv), [4:517) = 513 real tokens
#   (cls token at col 260), [517:524) right pad.
# Scan: tensor_tensor_scan per (e-tile, n): state = a*state + b along time,
#   a = r^(n+1) with r = exp(-delta); valid because A_log rows are
#   log(1..16) for every channel (asserted on host).
import sys
sys.path.insert(0, '/opt/trn_rl_repo')
import numpy as np
import ml_dtypes
from contextlib import ExitStack

import concourse.bass as bass
import concourse.tile as tile
from concourse import bacc, mybir
from concourse.bass_utils import run_bass_kernel_spmd

F32 = mybir.dt.float32
BF16 = mybir.dt.bfloat16
AF = mybir.ActivationFunctionType
ALU = mybir.AluOpType

B = 8; D = 768; E = 1536; N = 16; R = 48; DEPTH = 12; NCLS = 10
L = 513; MID = 256
C0 = 4                  # first real col
W = 524                 # padded width
CCLS = C0 + MID         # cls col = 260
ET = E // 128           # 12 e-tiles
DT = D // 128           # 6 d-tiles
M2 = 2 * ET             # 24 m-tiles of in_proj output
NSPLIT = 8              # gpsimd handles e-tiles >= NSPLIT
CHK = [(0, 262), (262, 524)]
PCO = (0, 512)          # psum col offset per chunk (bank aligned)


def ps2(ap):
    return ap.rearrange("p (c x) -> p c x", c=2)[:, :, 0:262]


def sb2(ap):
    return ap.rearrange("p (c x) -> p c x", c=2)


def build(nc, depth=DEPTH, sim_mode=False, debug_h=False):
    xp = nc.declare_dram_parameter("x_p", [2, 128, 512], F32, isOutput=False)
    pw = nc.declare_dram_parameter("pw_p", [2, 128, 768], BF16, isOutput=False)
    pbc = nc.declare_dram_parameter("pbc_p", [128, 12], F32, isOutput=False)
    inw = nc.declare_dram_parameter("inw_p", [DEPTH, 6, 128, 3072], BF16, isOutput=False)
    inb = nc.declare_dram_parameter("inb_p", [DEPTH, 128, 24], F32, isOutput=False)
    outw = nc.declare_dram_parameter("outw_p", [DEPTH, 2, 12, 128, 768], BF16, isOutput=False)
    xw = nc.declare_dram_parameter("xw_p", [DEPTH, 2, 12, 128, 112], BF16, isOutput=False)
    dtw = nc.declare_dram_parameter("dtw_p", [DEPTH, 2, 48, 1536], BF16, isOutput=False)
    pp = nc.declare_dram_parameter("pp_p", [DEPTH, 2, 128, ET, 8], F32, isOutput=False)
    hcon = nc.declare_dram_parameter("hcon_p", [128, 24], F32, isOutput=False)
    hwr = nc.declare_dram_parameter("hwr_p", [NCLS, 768], BF16, isOutput=False)
    hbr = nc.declare_dram_parameter("hbr_p", [NCLS, 1], F32, isOutput=False)
    ident = nc.declare_dram_parameter("ident_p", [128, 128], F32, isOutput=False)
    outp = nc.declare_dram_parameter("out", [NCLS], F32, isOutput=True)
    hdbg = nc.declare_dram_parameter("hdbg", [DT, 128, W], F32, isOutput=True) if debug_h else None

    with tile.TileContext(nc) as tc, ExitStack() as ctx:
        P = ctx.enter_context
        cpool = P(tc.tile_pool(name="const", bufs=1))
        hpool = P(tc.tile_pool(name="hres", bufs=1))
        spool = P(tc.tile_pool(name="stream", bufs=50))
        cvpool = P(tc.tile_pool(name="cv", bufs=4))
        apool = P(tc.tile_pool(name="astk", bufs=4))
        bpool = P(tc.tile_pool(name="bstk", bufs=1))
        hspool = P(tc.tile_pool(name="hstk", bufs=1))
        bcpool = P(tc.tile_pool(name="bcstk", bufs=2))
        wpool_in = P(tc.tile_pool(name="w_in", bufs=6))
        wpool_out = P(tc.tile_pool(name="w_out", bufs=3))
        wpool_x = P(tc.tile_pool(name="w_x", bufs=14))
        wpool_dt = P(tc.tile_pool(name="w_dt", bufs=1))
        wpool_pp = P(tc.tile_pool(name="w_pp", bufs=3))
        rpool = P(tc.tile_pool(name="rows", bufs=3))
        fpool = P(tc.tile_pool(name="f32b", bufs=3))
        dpool = P(tc.tile_pool(name="dram", bufs=2, space="DRAM"))
        psg = P(tc.tile_pool(name="psg", bufs=3, space="PSUM"))
        pss = P(tc.tile_pool(name="pss", bufs=2, space="PSUM"))

        def silu(out_ap, in_ap, bias=0.0):
            if not sim_mode:
                nc.scalar.activation(out_ap, in_ap, AF.Silu, bias=bias)
            else:
                p, f = out_ap.shape[0], in_ap.free_size()
                sg = cvpool.tile([128, W], BF16, tag="cv", name="sgt")
                pre = cvpool.tile([128, W], BF16, tag="cv", name="pret")
                nc.scalar.activation(sg[:p, :f], in_ap, AF.Sigmoid, bias=bias)
                nc.scalar.activation(pre[:p, :f], in_ap, AF.Identity, bias=bias)
                nc.vector.tensor_tensor(out=out_ap, in0=pre[:p, :f], in1=sg[:p, :f], op=ALU.mult)

        def softplus(out_ap, in_ap, bias):
            # ln(1 + exp(x + bias)) -- Softplus has no HW table
            p, f = out_ap.shape[0], in_ap.free_size()
            t = cvpool.tile([128, W], F32, tag="sp32", name="spt", bufs=2)
            nc.scalar.activation(t[:p, :f], in_ap, AF.Exp, bias=bias)
            nc.vector.tensor_scalar_add(t[:p, :f], t[:p, :f], 1.0)
            nc.scalar.activation(out_ap, t[:p, :f], AF.Ln)

        # constants
        ones_bf = cpool.tile([128, 1], BF16)
        nc.vector.memset(ones_bf[:], 1.0)
        eps_row = cpool.tile([1, 1], F32)
        nc.vector.memset(eps_row[:], 1e-5)
        ident_sb = cpool.tile([128, 128], F32)
        nc.sync.dma_start(ident_sb[:], ident[:])
        pbc_sb = cpool.tile([128, 12], F32)
        nc.sync.dma_start(pbc_sb[:], pbc[:])
        hcon_sb = cpool.tile([128, 24], F32)
        nc.sync.dma_start(hcon_sb[:], hcon[:])
        hw_sb = cpool.tile([NCLS, 768], BF16)
        nc.sync.dma_start(hw_sb[:], hwr[:])
        hb_sb = cpool.tile([NCLS, 1], F32)
        nc.sync.dma_start(hb_sb[:], hbr[:])

        # ---------------- patch embed ----------------
        h = []
        for m in range(DT):
            ht = hpool.tile([128, W], F32, tag=f"h{m}", name=f"hres{m}")
            h.append(ht)
        prhs = []
        for kt in range(2):
            pf = fpool.tile([128, 512], F32, tag="f32", name=f"pf{kt}")
            nc.sync.dma_start(pf[:], xp[kt])
            pb = spool.tile([128, 512], BF16, tag="s", name=f"pbt{kt}")
            nc.vector.tensor_copy(pb[:], pf[:])
            prhs.append(pb)
        pwt = []
        for kt in range(2):
            w_ = wpool_out.tile([128, 768], BF16, tag="wo", name=f"pwt{kt}")
            nc.sync.dma_start(w_[:], pw[kt])
            pwt.append(w_)
        for m in range(DT):
            ps = psg.tile([128, 524], F32, tag="g", name=f"pe_ps{m}")
            for kt in range(2):
                nc.tensor.matmul(ps[:, 0:512], pwt[kt][:, m * 128:(m + 1) * 128], prhs[kt][:],
                                 start=(kt == 0), stop=(kt == 1))
            nc.scalar.activation(h[m][:, C0:C0 + MID], ps[:, 0:MID], AF.Identity,
                                 bias=pbc_sb[:, m:m + 1])
            nc.scalar.activation(h[m][:, CCLS + 1:C0 + 1 + 512], ps[:, MID:512], AF.Identity,
                                 bias=pbc_sb[:, m:m + 1])
            nc.vector.tensor_copy(h[m][:, CCLS:CCLS + 1], pbc_sb[:, 6 + m:7 + m])
            nc.vector.memset(h[m][:, 0:C0], 0.0)
            nc.vector.memset(h[m][:, 517:W], 0.0)

        # ---------------- layers ----------------
        for li in range(depth):
            # ---- LayerNorm (stats via PE ones-matmul over partitions) ----
            hb, sq = [], []
            for m in range(DT):
                hbt = spool.tile([128, W], BF16, tag="s", name=f"hb{li}_{m}")
                nc.vector.tensor_copy(hbt[:], h[m][:])
                hb.append(hbt)
                sqt = spool.tile([128, W], BF16, tag="s", name=f"sq{li}_{m}")
                nc.scalar.activation(sqt[:], hbt[:], AF.Square)
                sq.append(sqt)
            srow = rpool.tile([1, W], F32, tag="r32", name=f"srow{li}")
            qrow = rpool.tile([1, W], F32, tag="r32", name=f"qrow{li}")
            for (c0, c1) in CHK:
                lp1 = pss.tile([1, 262], F32, tag="s", name=f"lnp1_{li}_{c0}")
                lp2 = pss.tile([1, 262], F32, tag="s", name=f"lnp2_{li}_{c0}")
                for m in range(DT):
                    nc.tensor.matmul(lp1[:, 0:c1 - c0], ones_bf[:], hb[m][:, c0:c1],
                                     start=(m == 0), stop=(m == DT - 1))
                for m in range(DT):
                    nc.tensor.matmul(lp2[:, 0:c1 - c0], ones_bf[:], sq[m][:, c0:c1],
                                     start=(m == 0), stop=(m == DT - 1))
                nc.scalar.activation(srow[:, c0:c1], lp1[:, 0:c1 - c0], AF.Copy, scale=1.0 / D)
                nc.scalar.activation(qrow[:, c0:c1], lp2[:, 0:c1 - c0], AF.Copy, scale=1.0 / D)
            m2row = rpool.tile([1, W], F32, tag="r32", name=f"m2r{li}")
            nc.scalar.activation(m2row[:], srow[:], AF.Square)
            nc.vector.tensor_tensor(out=qrow[:], in0=qrow[:], in1=m2row[:], op=ALU.subtract)
            nc.scalar.activation(m2row[:], qrow[:], AF.Ln, bias=eps_row[:, 0:1])
            rstd = rpool.tile([1, W], F32, tag="r32", name=f"rstd{li}")
            nc.scalar.activation(rstd[:], m2row[:], AF.Exp, scale=-0.5)
            mr = rpool.tile([1, W], F32, tag="r32", name=f"mr{li}")
            nc.vector.tensor_tensor(out=mr[:], in0=srow[:], in1=rstd[:], op=ALU.mult)
            Rb = fpool.tile([128, W], F32, tag="f32", name=f"Rb{li}")
            Mb = fpool.tile([128, W], F32, tag="f32", name=f"Mb{li}")
            nc.gpsimd.partition_broadcast(Rb[:], rstd[:])
            nc.gpsimd.partition_broadcast(Mb[:], mr[:])
            hn = []
            for m in range(DT):
                t = fpool.tile([128, W], F32, tag="f32", name=f"ht{li}_{m}")
                nc.vector.tensor_tensor(out=t[:], in0=h[m][:], in1=Rb[:], op=ALU.mult)
                hnt = spool.tile([128, W], BF16, tag="s", name=f"hn{li}_{m}")
                nc.vector.tensor_tensor(out=hnt[:], in0=t[:], in1=Mb[:], op=ALU.subtract)
                hn.append(hnt)

            # ---- in_proj GEMM ----
            inw_sb = []
            for k in range(6):
                wt = wpool_in.tile([128, 3072], BF16, tag="wi", name=f"inw{li}_{k}")
                nc.sync.dma_start(wt[:], inw[li, k])
                inw_sb.append(wt)
            inb_sb = wpool_pp.tile([128, 24], F32, tag="inb", name=f"inb{li}")
            nc.sync.dma_start(inb_sb[:], inb[li])
            xm, zs = [], []
            for m in range(M2):
                ps = psg.tile([128, 1024], F32, tag="g", name=f"xz_ps{li}_{m}")
                for ci, (c0, c1) in enumerate(CHK):
                    for k in range(6):
                        nc.tensor.matmul(ps[:, PCO[ci]:PCO[ci] + 262], inw_sb[k][:, m * 128:(m + 1) * 128],
                                         hn[k][:, c0:c1], start=(k == 0), stop=(k == 5))
                if m < ET:
                    t = spool.tile([128, W], BF16, tag="s", name=f"xm{li}_{m}")
                    nc.scalar.activation(sb2(t[:]), ps2(ps[:]), AF.Identity, bias=inb_sb[:, m:m + 1])
                    nc.vector.memset(t[:, 0:C0], 0.0)
                    nc.vector.memset(t[:, 517:W], 0.0)
                    xm.append(t)
                else:
                    t = spool.tile([128, W], BF16, tag="s", name=f"zs{li}_{m}")
                    silu(sb2(t[:]), ps2(ps[:]), bias=inb_sb[:, m:m + 1])
                    zs.append(t)

            # ---- conv (both dirs, silu fused) ----
            pps = []
            for s in range(2):
                t = wpool_pp.tile([128, ET, 8], F32, tag="pp", name=f"pp{li}_{s}")
                nc.sync.dma_start(t[:], pp[li, s])
                pps.append(t)
            u_f, u_b = [], []
            for s in range(2):
                for i in range(ET):
                    eng = nc.vector
                    wsl = pps[s][:, i, :]
                    c0t = cvpool.tile([128, W], BF16, tag="cv", name=f"cva{li}_{s}_{i}")
                    c1t = cvpool.tile([128, W], BF16, tag="cv", name=f"cvb{li}_{s}_{i}")
                    u = spool.tile([128, W], BF16, tag="s", name=f"u{li}_{s}_{i}")
                    if s == 0:
                        sl = [xm[i][:, 1:521], xm[i][:, 2:522], xm[i][:, 3:523], xm[i][:, 4:524]]
                        o0, o1 = 4, 524
                    else:
                        sl = [xm[i][:, 519:5:-1], xm[i][:, 518:4:-1], xm[i][:, 517:3:-1], xm[i][:, 516:2:-1]]
                        o0, o1 = 4, 518
                    eng.tensor_scalar_mul(c0t[:, o0:o1], sl[0], wsl[:, 0:1])
                    eng.scalar_tensor_tensor(c1t[:, o0:o1], sl[1], wsl[:, 1:2], c0t[:, o0:o1], ALU.mult, ALU.add)
                    eng.scalar_tensor_tensor(c0t[:, o0:o1], sl[2], wsl[:, 2:3], c1t[:, o0:o1], ALU.mult, ALU.add)
                    eng.scalar_tensor_tensor(c1t[:, o0:o1], sl[3], wsl[:, 3:4], c0t[:, o0:o1], ALU.mult, ALU.add)
                    silu(u[:, o0:o1], c1t[:, o0:o1], bias=wsl[:, 4:5])
                    nc.vector.memset(u[:, 0:C0], 0.0)
                    if s == 1:
                        nc.vector.memset(u[:, 518:W], 0.0)
                    (u_f if s == 0 else u_b).append(u)

            # ---- per-direction scan ----
            ya = []
            for s in range(2):
                u = u_f if s == 0 else u_b
                xw_sb = []
                for k in range(12):
                    t = wpool_x.tile([128, 112], BF16, tag="xw", name=f"xwt{li}_{s}_{k}")
                    nc.sync.dma_start(t[:], xw[li, s, k])
                    xw_sb.append(t)
                dtw_sb = wpool_dt.tile([48, 1536], BF16, tag="dtw", name=f"dtwt{li}_{s}")
                nc.sync.dma_start(dtw_sb[:], dtw[li, s])

                psd = psg.tile([128, 1024], F32, tag="g", name=f"dbc_ps{li}_{s}")
                for ci, (c0, c1) in enumerate(CHK):
                    for k in range(12):
                        nc.tensor.matmul(psd[0:112, PCO[ci]:PCO[ci] + 262], xw_sb[k][:], u[k][:, c0:c1],
                                         start=(k == 0), stop=(k == 11))
                dbc = spool.tile([128, W], BF16, tag="s", name=f"dbc{li}_{s}")
                nc.scalar.activation(sb2(dbc[0:112, :]), ps2(psd[0:112, :]), AF.Copy)
                # B/C broadcast stacks via DMA replication
                Bb = bcpool.tile([128, N, W], BF16, tag="bc", name=f"Bb{li}_{s}")
                Cb = bcpool.tile([128, N, W], BF16, tag="bc", name=f"Cb{li}_{s}")
                drows = dpool.tile([32, W], BF16, tag="dr", name=f"drows{li}_{s}")
                nc.sync.dma_start(drows[0:16, :], dbc[64:80, :])
                nc.sync.dma_start(drows[16:32, :], dbc[96:112, :])
                for n in range(N):
                    nc.sync.dma_start(Bb[:, n, :], drows[n:n + 1, :].broadcast_to([128, W]))
                    nc.sync.dma_start(Cb[:, n, :], drows[16 + n:17 + n, :].broadcast_to([128, W]))

                for i in range(ET):
                    eng = nc.vector if i < 6 else nc.gpsimd
                    # delta GEMM + softplus
                    psdt = psg.tile([128, 1024], F32, tag="g", name=f"dt_ps{li}_{s}_{i}")
                    for ci, (c0, c1) in enumerate(CHK):
                        nc.tensor.matmul(psdt[:, PCO[ci]:PCO[ci] + 262], dtw_sb[:, i * 128:(i + 1) * 128],
                                         dbc[0:48, c0:c1], start=True, stop=True)
                    dl = spool.tile([128, W], BF16, tag="s", name=f"dl{li}_{s}_{i}")
                    softplus(sb2(dl[:]), ps2(psdt[:]), bias=pps[s][:, i, 5:6])
                    g_ = spool.tile([128, W], BF16, tag="s", name=f"g{li}_{s}_{i}")
                    eng.tensor_tensor(out=g_[:], in0=dl[:], in1=u[i][:], op=ALU.mult)
                    nc.vector.memset(g_[:, 0:C0], 0.0)
                    nc.vector.memset(g_[:, 517:W], 0.0)
                    ysi = spool.tile([128, W], BF16, tag="s", name=f"ys{li}_{s}_{i}")
                    aq = [None] * 4
                    for q in range(4):
                        ast = apool.tile([128, 4, W], BF16, tag="a", name=f"a{li}_{s}_{i}_{q}")
                        aq[q] = ast
                        if q == 0:
                            nc.scalar.activation(ast[:, 0, :], dl[:], AF.Exp, scale=-1.0)
                            nc.scalar.activation(ast[:, 1, :], ast[:, 0, :], AF.Square)
                            eng.tensor_tensor(out=ast[:, 2, :], in0=ast[:, 1, :], in1=ast[:, 0, :], op=ALU.mult)
                            nc.scalar.activation(ast[:, 3, :], ast[:, 1, :], AF.Square)
                        elif q == 1:
                            for j in range(3):
                                eng.tensor_tensor(out=ast[:, j, :], in0=aq[0][:, 3, :], in1=aq[0][:, j, :], op=ALU.mult)
                            nc.scalar.activation(ast[:, 3, :], aq[0][:, 3, :], AF.Square)
                        elif q == 2:
                            for j in range(4):
                                eng.tensor_tensor(out=ast[:, j, :], in0=aq[1][:, 3, :], in1=aq[0][:, j, :], op=ALU.mult)
                        else:
                            for j in range(3):
                                eng.tensor_tensor(out=ast[:, j, :], in0=aq[1][:, 3, :], in1=aq[1][:, j, :], op=ALU.mult)
                            nc.scalar.activation(ast[:, 3, :], aq[1][:, 3, :], AF.Square)
                        bst = bpool.tile([128, 4, W], BF16, tag="b", name=f"b{li}_{s}_{i}_{q}")
                        eng.tensor_tensor(out=bst[:], in0=g_[:].unsqueeze(1).broadcast_to([128, 4, W]),
                                          in1=Bb[:, q * 4:(q + 1) * 4, :], op=ALU.mult)
                        hst = hspool.tile([128, 4, W], BF16, tag="h", name=f"hs{li}_{s}_{i}_{q}")
                        for n in range(4):
                            nc.vector.tensor_tensor_scan(hst[:, n, :], ast[:, n, :], bst[:, n, :],
                                                         0.0, ALU.mult, ALU.add)
                        eng.tensor_tensor(out=hst[:], in0=hst[:], in1=Cb[:, q * 4:(q + 1) * 4, :], op=ALU.mult)
                        eng.tensor_tensor(out=hst[:, 0:2, :], in0=hst[:, 0:2, :], in1=hst[:, 2:4, :], op=ALU.add)
                        if q == 0:
                            eng.tensor_tensor(out=ysi[:], in0=hst[:, 0, :], in1=hst[:, 1, :], op=ALU.add)
                        else:
                            eng.tensor_tensor(out=hst[:, 0, :], in0=hst[:, 0, :], in1=hst[:, 1, :], op=ALU.add)
                            eng.tensor_tensor(out=ysi[:], in0=ysi[:], in1=hst[:, 0, :], op=ALU.add)
                    if s == 0:
                        yat = spool.tile([128, W], BF16, tag="s", name=f"ya{li}_{i}")
                        nc.vector.scalar_tensor_tensor(yat[:], u[i][:], pps[s][:, i, 6:7], ysi[:], ALU.mult, ALU.add)
                        ya.append(yat)
                    else:
                        nc.vector.scalar_tensor_tensor(ysi[:], u[i][:], pps[s][:, i, 6:7], ysi[:], ALU.mult, ALU.add)
                        eng.tensor_tensor(out=ya[i][:, 4:518], in0=ya[i][:, 4:518],
                                          in1=ysi[:, 516:2:-1], op=ALU.add)
                        eng.tensor_tensor(out=ya[i][:], in0=ya[i][:], in1=zs[i][:], op=ALU.mult)

            # ---- out_proj GEMM + residual (k-streamed, 3 m per group) ----
            for mg in range(2):
                psm = [psg.tile([128, 1024], F32, tag="g", name=f"op_ps{li}_{mg}_{m}") for m in range(3)]
                for k in range(12):
                    wt = wpool_out.tile([128, 768], BF16, tag="wo", name=f"outw{li}_{mg}_{k}")
                    nc.sync.dma_start(wt[:], outw[li, mg, k])
                    for m in range(3):
                        mm = mg * 3 + m
                        for ci, (c0, c1) in enumerate(CHK):
                            nc.tensor.matmul(psm[m][:, PCO[ci]:PCO[ci] + 262], wt[:, mm * 128:(mm + 1) * 128],
                                             ya[k][:, c0:c1], start=(k == 0), stop=(k == 11))
                for m in range(3):
                    mm = mg * 3 + m
                    nc.vector.tensor_tensor(out=sb2(h[mm][:]), in0=sb2(h[mm][:]), in1=ps2(psm[m][:]), op=ALU.add)

        if debug_h:
            for m in range(DT):
                nc.sync.dma_start(hdbg[m], h[m][:])

        # ---------------- head ----------------
        frow = rpool.tile([1, 768], F32, tag="hrow", name="frow", bufs=4)
        for m in range(DT):
            fps = pss.tile([1, 262], F32, tag="s", name=f"fps{m}")
            nc.tensor.matmul(fps[:, 0:128], h[m][:, CCLS:CCLS + 1], ident_sb[:],
                             start=True, stop=True)
            nc.scalar.copy(frow[:, m * 128:(m + 1) * 128], fps[:, 0:128])

        def ln_row(row, wcol, bcol, nm):
            mrow = rpool.tile([1, 1], F32, tag="h1", name=f"m_{nm}", bufs=8)
            nc.vector.tensor_reduce(mrow[:], row[:], mybir.AxisListType.X, ALU.add)
            nc.vector.tensor_scalar_mul(mrow[:], mrow[:], 1.0 / D)
            cen = rpool.tile([1, 768], F32, tag="hrow", name=f"c_{nm}", bufs=4)
            nc.vector.tensor_scalar_sub(cen[:], row[:], mrow[:, 0:1])
            sq_ = rpool.tile([1, 768], F32, tag="hrow", name=f"q_{nm}", bufs=4)
            nc.scalar.activation(sq_[:], cen[:], AF.Square)
            vrow = rpool.tile([1, 1], F32, tag="h1", name=f"v_{nm}", bufs=8)
            nc.vector.tensor_reduce(vrow[:], sq_[:], mybir.AxisListType.X, ALU.add)
            nc.vector.tensor_scalar_mul(vrow[:], vrow[:], 1.0 / D)
            st = rpool.tile([1, 1], F32, tag="h1", name=f"st_{nm}", bufs=8)
            nc.scalar.activation(st[:], vrow[:], AF.Ln, bias=eps_row[:, 0:1])
            rs = rpool.tile([1, 1], F32, tag="h1", name=f"rs_{nm}", bufs=8)
            nc.scalar.activation(rs[:], st[:], AF.Exp, scale=-0.5)
            nrm = rpool.tile([1, 768], F32, tag="hrow", name=f"n_{nm}", bufs=4)
            nc.vector.tensor_scalar_mul(nrm[:], cen[:], rs[:, 0:1])
            wrow = rpool.tile([1, 768], F32, tag="hrow", name=f"w_{nm}", bufs=4)
            brow = rpool.tile([1, 768], F32, tag="hrow", name=f"b_{nm}", bufs=4)
            for m in range(DT):
                fps1 = pss.tile([1, 262], F32, tag="s", name=f"w_ps_{nm}_{m}")
                nc.tensor.matmul(fps1[:, 0:128], hcon_sb[:, wcol * 6 + m:wcol * 6 + m + 1],
                                 ident_sb[:], start=True, stop=True)
                nc.scalar.copy(wrow[:, m * 128:(m + 1) * 128], fps1[:, 0:128])
                fps2 = pss.tile([1, 262], F32, tag="s", name=f"b_ps_{nm}_{m}")
                nc.tensor.matmul(fps2[:, 0:128], hcon_sb[:, bcol * 6 + m:bcol * 6 + m + 1],
                                 ident_sb[:], start=True, stop=True)
                nc.scalar.copy(brow[:, m * 128:(m + 1) * 128], fps2[:, 0:128])
            o1 = rpool.tile([1, 768], F32, tag="hrow", name=f"o_{nm}", bufs=4)
            nc.vector.tensor_tensor(out=o1[:], in0=nrm[:], in1=wrow[:], op=ALU.mult)
            nc.vector.tensor_tensor(out=o1[:], in0=o1[:], in1=brow[:], op=ALU.add)
            return o1

        f1 = ln_row(frow, 0, 1, "ln1")
        f2 = ln_row(f1, 2, 3, "ln2")
        f2b = rpool.tile([1, 768], BF16, tag="hrow", name="f2b", bufs=4)
        nc.vector.tensor_copy(f2b[:], f2[:])
        fb2 = rpool.tile([NCLS, 768], BF16, tag="hbc", name="fbc", bufs=2)
        fdr = dpool.tile([1, 768], BF16, tag="fdr", name="fdr")
        nc.sync.dma_start(fdr[:], f2b[:])
        nc.sync.dma_start(fb2[:], fdr[:].broadcast_to([NCLS, 768]))
        pr = rpool.tile([NCLS, 768], BF16, tag="hbc", name="pr", bufs=2)
        nc.vector.tensor_tensor(out=pr[:], in0=fb2[:], in1=hw_sb[:], op=ALU.mult)
        lg = rpool.tile([NCLS, 1], F32, tag="h1", name="lg", bufs=8)
        nc.vector.tensor_reduce(lg[:], pr[:], mybir.AxisListType.X, ALU.add)
        nc.vector.tensor_tensor(out=lg[:], in0=lg[:], in1=hb_sb[:], op=ALU.add)
        nc.sync.dma_start(outp[:].unsqueeze(-1), lg[:])
    return nc


# ---------------- host side ----------------
_cache = {}


def _prep(inputs, depth=DEPTH):
    f32 = np.float32
    bf = ml_dtypes.bfloat16
    g = lambda k: np.asarray(inputs[k], f32)
    assert np.allclose(np.exp(g('A_log_f')), np.arange(1, N + 1, dtype=f32)[None, None, :], atol=1e-4), "A structure"
    assert np.allclose(np.exp(g('A_log_b')), np.arange(1, N + 1, dtype=f32)[None, None, :], atol=1e-4), "A structure"

    w = {}
    pwf = g('patch_w').reshape(D, 256)
    lhsT = pwf.T.astype(bf)
    w['pw_p'] = np.stack([lhsT[0:128], lhsT[128:256]])
    pbcv = np.zeros((128, 12), f32)
    for m in range(DT):
        pbcv[:, m] = g('patch_b')[m * 128:(m + 1) * 128]
        pbcv[:, 6 + m] = g('cls_token').reshape(D)[m * 128:(m + 1) * 128]
    w['pbc_p'] = pbcv

    inw_l, inb_l, outw_l, xw_l, dtw_l, pp_l = [], [], [], [], [], []
    for i in range(depth):
        Wf = (g('in_proj_w')[i] * g('norm_w')[i][None, :]).T.astype(bf)
        inw_l.append(np.stack([Wf[k * 128:(k + 1) * 128] for k in range(6)]))
        ib = (g('in_proj_w')[i] @ g('norm_b')[i]).astype(f32)
        inb_l.append(ib.reshape(24, 128).T.copy())
        oT = g('out_proj_w')[i].T.astype(bf)   # [1536, 768]
        ow = np.stack([oT[k * 128:(k + 1) * 128] for k in range(12)])
        outw_l.append(np.stack([ow, ow]))      # same for both m-groups
        xw_s, dtw_s, pp_s = [], [], []
        for s, sfx in enumerate(('f', 'b')):
            xpr = g('x_proj_w_' + sfx)[i]
            xe = np.zeros((112, E), f32)
            xe[0:48] = xpr[0:48]; xe[64:80] = xpr[48:64]; xe[96:112] = xpr[64:80]
            xeT = xe.T.astype(bf)
            xw_s.append(np.stack([xeT[k * 128:(k + 1) * 128] for k in range(12)]))
            dtw_s.append(g('dt_proj_w_' + sfx)[i].T.astype(bf))
            ppv = np.zeros((128, ET, 8), f32)
            for it in range(ET):
                sl = slice(it * 128, (it + 1) * 128)
                ppv[:, it, 0:4] = g('conv_w_' + sfx)[i][sl]
                ppv[:, it, 4] = g('conv_b_' + sfx)[i][sl]
                ppv[:, it, 5] = g('dt_proj_b_' + sfx)[i][sl]
                ppv[:, it, 6] = g('D_' + sfx)[i][sl]
            pp_s.append(ppv)
        xw_l.append(np.stack(xw_s)); dtw_l.append(np.stack(dtw_s)); pp_l.append(np.stack(pp_s))
    w['inw_p'] = np.stack(inw_l).astype(bf); w['inb_p'] = np.stack(inb_l)
    w['outw_p'] = np.stack(outw_l).astype(bf); w['xw_p'] = np.stack(xw_l).astype(bf)
    w['dtw_p'] = np.stack(dtw_l).astype(bf); w['pp_p'] = np.stack(pp_l)
    if depth < DEPTH:
        for k in ('inw_p', 'inb_p', 'outw_p', 'xw_p', 'dtw_p', 'pp_p'):
            pad = [(0, DEPTH - depth)] + [(0, 0)] * (w[k].ndim - 1)
            w[k] = np.pad(w[k], pad)

    hc = np.zeros((128, 24), f32)
    for m in range(DT):
        sl = slice(m * 128, (m + 1) * 128)
        hc[:, m] = g('fnorm_w')[sl]; hc[:, 6 + m] = g('fnorm_b')[sl]
        hc[:, 12 + m] = g('head_norm_w')[sl]; hc[:, 18 + m] = g('head_norm_b')[sl]
    w['hcon_p'] = hc
    w['hwr_p'] = g('head_w').astype(bf)
    w['hbr_p'] = g('head_b').reshape(NCLS, 1)
    w['ident_p'] = np.eye(128, dtype=f32)
    return w


def _prep_x(x):
    xs = []
    for c in range(x.shape[0]):
        xc = np.asarray(x[c, 0], np.float32)
        p = xc.reshape(8, 16, 64, 16).transpose(1, 3, 0, 2).reshape(256, 512)
        xs.append(np.stack([p[0:128], p[128:256]]).copy())
    return xs


def kernel(**inputs):
    if 'nc' not in _cache:
        nc = bacc.Bacc("TRN2", target_bir_lowering=False, debug=False, num_devices=B)
        build(nc, DEPTH, sim_mode=False)
        nc.compile()
        _cache['nc'] = nc
    nc = _cache['nc']
    w = _prep(inputs, DEPTH)
    xs = _prep_x(np.asarray(inputs['x']))
    in_maps = []
    for c in range(B):
        m = dict(w)
        m['x_p'] = xs[c]
        in_maps.append(m)
    res = run_bass_kernel_spmd(nc, in_maps, core_ids=list(range(B)))
    return np.stack([np.asarray(res.results[c]['out'], np.float32) for c in range(B)])


# revision 14
# speedup vs baseline: 1.2003x; 1.2003x over previous

# AudioMamba (bidirectional Mamba encoder) on 8 TRN2 NeuronCores.
# Sharding: data-parallel over batch (1 sample per core), no collectives.
# Layout: feature-major [channels-on-partitions, time-on-free], W=524 cols:
#   cols [0:4) left pad (zeros for causal conv), [4:517) = 513 real tokens
#   (cls token at col 260), [517:524) right pad.
# Scan: tensor_tensor_scan per (e-tile, n): state = a*state + b along time,
#   a = r^(n+1) with r = exp(-delta); valid because A_log rows are
#   log(1..16) for every channel (asserted on host).
import sys
sys.path.insert(0, '/opt/trn_rl_repo')
import numpy as np
import ml_dtypes
from contextlib import ExitStack

import concourse.bass as bass
import concourse.tile as tile
from concourse import bacc, mybir
from concourse.bass_utils import run_bass_kernel_spmd

F32 = mybir.dt.float32
BF16 = mybir.dt.bfloat16
AF = mybir.ActivationFunctionType
ALU = mybir.AluOpType

B = 8; D = 768; E = 1536; N = 16; R = 48; DEPTH = 12; NCLS = 10
L = 513; MID = 256
C0 = 4                  # first real col
W = 524                 # padded width
WP = 526                # stack pitch (4B-aligned bf16 slices per n)
CCLS = C0 + MID         # cls col = 260
ET = E // 128           # 12 e-tiles
DT = D // 128           # 6 d-tiles
M2 = 2 * ET             # 24 m-tiles of in_proj output
NSPLIT = 8              # gpsimd handles e-tiles >= NSPLIT
CHK = [(0, 262), (262, 524)]
PCO = (0, 512)          # psum col offset per chunk (bank aligned)


def ps2(ap):
    return ap.rearrange("p (c x) -> p c x", c=2)[:, :, 0:262]


def sb2(ap):
    return ap.rearrange("p (c x) -> p c x", c=2)


def build(nc, depth=DEPTH, sim_mode=False, debug_h=False):
    xp = nc.declare_dram_parameter("x_p", [2, 128, 512], F32, isOutput=False)
    pw = nc.declare_dram_parameter("pw_p", [2, 128, 768], BF16, isOutput=False)
    pbc = nc.declare_dram_parameter("pbc_p", [128, 12], F32, isOutput=False)
    inw = nc.declare_dram_parameter("inw_p", [DEPTH, 6, 128, 3072], BF16, isOutput=False)
    inb = nc.declare_dram_parameter("inb_p", [DEPTH, 128, 24], F32, isOutput=False)
    outw = nc.declare_dram_parameter("outw_p", [DEPTH, 2, 12, 128, 768], BF16, isOutput=False)
    xw = nc.declare_dram_parameter("xw_p", [DEPTH, 2, 12, 128, 112], BF16, isOutput=False)
    dtw = nc.declare_dram_parameter("dtw_p", [DEPTH, 2, 48, 1536], BF16, isOutput=False)
    pp = nc.declare_dram_parameter("pp_p", [DEPTH, 2, 128, ET, 8], F32, isOutput=False)
    hcon = nc.declare_dram_parameter("hcon_p", [128, 24], F32, isOutput=False)
    hwr = nc.declare_dram_parameter("hwr_p", [NCLS, 768], BF16, isOutput=False)
    hbr = nc.declare_dram_parameter("hbr_p", [NCLS, 1], F32, isOutput=False)
    ident = nc.declare_dram_parameter("ident_p", [128, 128], F32, isOutput=False)
    outp = nc.declare_dram_parameter("out", [NCLS], F32, isOutput=True)
    hdbg = nc.declare_dram_parameter("hdbg", [DT, 128, W], F32, isOutput=True) if debug_h else None

    with tile.TileContext(nc) as tc, ExitStack() as ctx:
        P = ctx.enter_context
        cpool = P(tc.tile_pool(name="const", bufs=1))
        hpool = P(tc.tile_pool(name="hres", bufs=1))
        spool = P(tc.tile_pool(name="stream", bufs=50))
        cvpool = P(tc.tile_pool(name="cv", bufs=4))
        apool = P(tc.tile_pool(name="astk", bufs=4))
        bpool = P(tc.tile_pool(name="bstk", bufs=1))
        hspool = P(tc.tile_pool(name="hstk", bufs=1))
        bcpool = P(tc.tile_pool(name="bcstk", bufs=2))
        wpool_in = P(tc.tile_pool(name="w_in", bufs=6))
        wpool_out = P(tc.tile_pool(name="w_out", bufs=3))
        wpool_x = P(tc.tile_pool(name="w_x", bufs=14))
        wpool_dt = P(tc.tile_pool(name="w_dt", bufs=1))
        wpool_pp = P(tc.tile_pool(name="w_pp", bufs=3))
        rpool = P(tc.tile_pool(name="rows", bufs=3))
        fpool = P(tc.tile_pool(name="f32b", bufs=3))
        dpool = P(tc.tile_pool(name="dram", bufs=2, space="DRAM"))
        psg = P(tc.tile_pool(name="psg", bufs=3, space="PSUM"))
        pss = P(tc.tile_pool(name="pss", bufs=2, space="PSUM"))

        def silu(out_ap, in_ap, bias=0.0):
            if not sim_mode:
                nc.scalar.activation(out_ap, in_ap, AF.Silu, bias=bias)
            else:
                p, f = out_ap.shape[0], in_ap.free_size()
                sg = cvpool.tile([128, W], BF16, tag="cv", name="sgt")
                pre = cvpool.tile([128, W], BF16, tag="cv", name="pret")
                nc.scalar.activation(sg[:p, :f], in_ap, AF.Sigmoid, bias=bias)
                nc.scalar.activation(pre[:p, :f], in_ap, AF.Identity, bias=bias)
                nc.vector.tensor_tensor(out=out_ap, in0=pre[:p, :f], in1=sg[:p, :f], op=ALU.mult)

        def softplus(out_ap, in_ap, bias):
            # ln(1 + exp(x + bias)) -- Softplus has no HW table
            p, f = out_ap.shape[0], in_ap.free_size()
            t = cvpool.tile([128, W], F32, tag="sp32", name="spt", bufs=2)
            nc.scalar.activation(t[:p, :f], in_ap, AF.Exp, bias=bias)
            nc.vector.tensor_scalar_add(t[:p, :f], t[:p, :f], 1.0)
            nc.scalar.activation(out_ap, t[:p, :f], AF.Ln)

        # constants
        ones_bf = cpool.tile([128, 1], BF16)
        nc.vector.memset(ones_bf[:], 1.0)
        eps_row = cpool.tile([1, 1], F32)
        nc.vector.memset(eps_row[:], 1e-5)
        ident_sb = cpool.tile([128, 128], F32)
        nc.sync.dma_start(ident_sb[:], ident[:])
        pbc_sb = cpool.tile([128, 12], F32)
        nc.sync.dma_start(pbc_sb[:], pbc[:])
        hcon_sb = cpool.tile([128, 24], F32)
        nc.sync.dma_start(hcon_sb[:], hcon[:])
        hw_sb = cpool.tile([NCLS, 768], BF16)
        nc.sync.dma_start(hw_sb[:], hwr[:])
        hb_sb = cpool.tile([NCLS, 1], F32)
        nc.sync.dma_start(hb_sb[:], hbr[:])

        # ---------------- patch embed ----------------
        h = []
        for m in range(DT):
            ht = hpool.tile([128, W], F32, tag=f"h{m}", name=f"hres{m}")
            h.append(ht)
        prhs = []
        for kt in range(2):
            pf = fpool.tile([128, 512], F32, tag="f32", name=f"pf{kt}")
            nc.sync.dma_start(pf[:], xp[kt])
            pb = spool.tile([128, 512], BF16, tag="s", name=f"pbt{kt}")
            nc.vector.tensor_copy(pb[:], pf[:])
            prhs.append(pb)
        pwt = []
        for kt in range(2):
            w_ = wpool_out.tile([128, 768], BF16, tag="wo", name=f"pwt{kt}")
            nc.sync.dma_start(w_[:], pw[kt])
            pwt.append(w_)
        for m in range(DT):
            ps = psg.tile([128, 524], F32, tag="g", name=f"pe_ps{m}")
            for kt in range(2):
                nc.tensor.matmul(ps[:, 0:512], pwt[kt][:, m * 128:(m + 1) * 128], prhs[kt][:],
                                 start=(kt == 0), stop=(kt == 1))
            nc.scalar.activation(h[m][:, C0:C0 + MID], ps[:, 0:MID], AF.Identity,
                                 bias=pbc_sb[:, m:m + 1])
            nc.scalar.activation(h[m][:, CCLS + 1:C0 + 1 + 512], ps[:, MID:512], AF.Identity,
                                 bias=pbc_sb[:, m:m + 1])
            nc.vector.tensor_copy(h[m][:, CCLS:CCLS + 1], pbc_sb[:, 6 + m:7 + m])
            nc.vector.memset(h[m][:, 0:C0], 0.0)
            nc.vector.memset(h[m][:, 517:W], 0.0)

        # ---------------- layers ----------------
        for li in range(depth):
            # ---- LayerNorm (stats via PE ones-matmul over partitions) ----
            hb, sq = [], []
            for m in range(DT):
                hbt = spool.tile([128, W], BF16, tag="s", name=f"hb{li}_{m}")
                nc.vector.tensor_copy(hbt[:], h[m][:])
                hb.append(hbt)
                sqt = spool.tile([128, W], BF16, tag="s", name=f"sq{li}_{m}")
                nc.scalar.activation(sqt[:], hbt[:], AF.Square)
                sq.append(sqt)
            srow = rpool.tile([1, W], F32, tag="r32", name=f"srow{li}")
            qrow = rpool.tile([1, W], F32, tag="r32", name=f"qrow{li}")
            for (c0, c1) in CHK:
                lp1 = pss.tile([1, 262], F32, tag="s", name=f"lnp1_{li}_{c0}")
                lp2 = pss.tile([1, 262], F32, tag="s", name=f"lnp2_{li}_{c0}")
                for m in range(DT):
                    nc.tensor.matmul(lp1[:, 0:c1 - c0], ones_bf[:], hb[m][:, c0:c1],
                                     start=(m == 0), stop=(m == DT - 1))
                for m in range(DT):
                    nc.tensor.matmul(lp2[:, 0:c1 - c0], ones_bf[:], sq[m][:, c0:c1],
                                     start=(m == 0), stop=(m == DT - 1))
                nc.scalar.activation(srow[:, c0:c1], lp1[:, 0:c1 - c0], AF.Copy, scale=1.0 / D)
                nc.scalar.activation(qrow[:, c0:c1], lp2[:, 0:c1 - c0], AF.Copy, scale=1.0 / D)
            m2row = rpool.tile([1, W], F32, tag="r32", name=f"m2r{li}")
            nc.scalar.activation(m2row[:], srow[:], AF.Square)
            nc.vector.tensor_tensor(out=qrow[:], in0=qrow[:], in1=m2row[:], op=ALU.subtract)
            nc.scalar.activation(m2row[:], qrow[:], AF.Ln, bias=eps_row[:, 0:1])
            rstd = rpool.tile([1, W], F32, tag="r32", name=f"rstd{li}")
            nc.scalar.activation(rstd[:], m2row[:], AF.Exp, scale=-0.5)
            mr = rpool.tile([1, W], F32, tag="r32", name=f"mr{li}")
            nc.vector.tensor_tensor(out=mr[:], in0=srow[:], in1=rstd[:], op=ALU.mult)
            Rb = fpool.tile([128, W], F32, tag="f32", name=f"Rb{li}")
            Mb = fpool.tile([128, W], F32, tag="f32", name=f"Mb{li}")
            nc.gpsimd.partition_broadcast(Rb[:], rstd[:])
            nc.gpsimd.partition_broadcast(Mb[:], mr[:])
            hn = []
            for m in range(DT):
                t = fpool.tile([128, W], F32, tag="f32", name=f"ht{li}_{m}")
                nc.vector.tensor_tensor(out=t[:], in0=h[m][:], in1=Rb[:], op=ALU.mult)
                hnt = spool.tile([128, W], BF16, tag="s", name=f"hn{li}_{m}")
                nc.vector.tensor_tensor(out=hnt[:], in0=t[:], in1=Mb[:], op=ALU.subtract)
                hn.append(hnt)

            # ---- in_proj GEMM ----
            inw_sb = []
            for k in range(6):
                wt = wpool_in.tile([128, 3072], BF16, tag="wi", name=f"inw{li}_{k}")
                nc.sync.dma_start(wt[:], inw[li, k])
                inw_sb.append(wt)
            inb_sb = wpool_pp.tile([128, 24], F32, tag="inb", name=f"inb{li}")
            nc.sync.dma_start(inb_sb[:], inb[li])
            xm, zs = [], []
            for m in range(M2):
                ps = psg.tile([128, 1024], F32, tag="g", name=f"xz_ps{li}_{m}")
                for ci, (c0, c1) in enumerate(CHK):
                    for k in range(6):
                        nc.tensor.matmul(ps[:, PCO[ci]:PCO[ci] + 262], inw_sb[k][:, m * 128:(m + 1) * 128],
                                         hn[k][:, c0:c1], start=(k == 0), stop=(k == 5))
                if m < ET:
                    t = spool.tile([128, W], BF16, tag="s", name=f"xm{li}_{m}")
                    nc.scalar.activation(sb2(t[:]), ps2(ps[:]), AF.Identity, bias=inb_sb[:, m:m + 1])
                    nc.vector.memset(t[:, 0:C0], 0.0)
                    nc.vector.memset(t[:, 517:W], 0.0)
                    xm.append(t)
                else:
                    t = spool.tile([128, W], BF16, tag="s", name=f"zs{li}_{m}")
                    silu(sb2(t[:]), ps2(ps[:]), bias=inb_sb[:, m:m + 1])
                    zs.append(t)

            # ---- conv (both dirs, silu fused) ----
            pps = []
            for s in range(2):
                t = wpool_pp.tile([128, ET, 8], F32, tag="pp", name=f"pp{li}_{s}")
                nc.sync.dma_start(t[:], pp[li, s])
                pps.append(t)
            u_f, u_b = [], []
            for s in range(2):
                for i in range(ET):
                    eng = nc.vector
                    wsl = pps[s][:, i, :]
                    c0t = cvpool.tile([128, W], BF16, tag="cv", name=f"cva{li}_{s}_{i}")
                    c1t = cvpool.tile([128, W], BF16, tag="cv", name=f"cvb{li}_{s}_{i}")
                    u = spool.tile([128, W], BF16, tag="s", name=f"u{li}_{s}_{i}")
                    if s == 0:
                        sl = [xm[i][:, 1:521], xm[i][:, 2:522], xm[i][:, 3:523], xm[i][:, 4:524]]
                        o0, o1 = 4, 524
                    else:
                        sl = [xm[i][:, 519:5:-1], xm[i][:, 518:4:-1], xm[i][:, 517:3:-1], xm[i][:, 516:2:-1]]
                        o0, o1 = 4, 518
                    eng.tensor_scalar_mul(c0t[:, o0:o1], sl[0], wsl[:, 0:1])
                    eng.scalar_tensor_tensor(c1t[:, o0:o1], sl[1], wsl[:, 1:2], c0t[:, o0:o1], ALU.mult, ALU.add)
                    eng.scalar_tensor_tensor(c0t[:, o0:o1], sl[2], wsl[:, 2:3], c1t[:, o0:o1], ALU.mult, ALU.add)
                    eng.scalar_tensor_tensor(c1t[:, o0:o1], sl[3], wsl[:, 3:4], c0t[:, o0:o1], ALU.mult, ALU.add)
                    silu(u[:, o0:o1], c1t[:, o0:o1], bias=wsl[:, 4:5])
                    nc.vector.memset(u[:, 0:C0], 0.0)
                    if s == 1:
                        nc.vector.memset(u[:, 518:W], 0.0)
                    (u_f if s == 0 else u_b).append(u)

            # ---- per-direction scan ----
            ya = []
            for s in range(2):
                u = u_f if s == 0 else u_b
                xw_sb = []
                for k in range(12):
                    t = wpool_x.tile([128, 112], BF16, tag="xw", name=f"xwt{li}_{s}_{k}")
                    nc.sync.dma_start(t[:], xw[li, s, k])
                    xw_sb.append(t)
                dtw_sb = wpool_dt.tile([48, 1536], BF16, tag="dtw", name=f"dtwt{li}_{s}")
                nc.sync.dma_start(dtw_sb[:], dtw[li, s])

                psd = psg.tile([128, 1024], F32, tag="g", name=f"dbc_ps{li}_{s}")
                for ci, (c0, c1) in enumerate(CHK):
                    for k in range(12):
                        nc.tensor.matmul(psd[0:112, PCO[ci]:PCO[ci] + 262], xw_sb[k][:], u[k][:, c0:c1],
                                         start=(k == 0), stop=(k == 11))
                dbc = spool.tile([128, W], BF16, tag="s", name=f"dbc{li}_{s}")
                nc.scalar.activation(sb2(dbc[0:112, :]), ps2(psd[0:112, :]), AF.Copy)
                # B/C broadcast stacks via DMA replication
                Bbf = bcpool.tile([128, N, WP], BF16, tag="bc", name=f"Bb{li}_{s}")
                Bb = Bbf[:, :, 0:W]
                Cbf = bcpool.tile([128, N, WP], BF16, tag="bc", name=f"Cb{li}_{s}")
                Cb = Cbf[:, :, 0:W]
                drows = dpool.tile([32, W], BF16, tag="dr", name=f"drows{li}_{s}")
                nc.sync.dma_start(drows[0:16, :], dbc[64:80, :])
                nc.sync.dma_start(drows[16:32, :], dbc[96:112, :])
                for n in range(N):
                    nc.sync.dma_start(Bb[:, n, :], drows[n:n + 1, :].broadcast_to([128, W]))
                    nc.sync.dma_start(Cb[:, n, :], drows[16 + n:17 + n, :].broadcast_to([128, W]))

                for i in range(ET):
                    eng = nc.vector if i < 9 else nc.gpsimd
                    # delta GEMM + softplus
                    psdt = psg.tile([128, 1024], F32, tag="g", name=f"dt_ps{li}_{s}_{i}")
                    for ci, (c0, c1) in enumerate(CHK):
                        nc.tensor.matmul(psdt[:, PCO[ci]:PCO[ci] + 262], dtw_sb[:, i * 128:(i + 1) * 128],
                                         dbc[0:48, c0:c1], start=True, stop=True)
                    dl = spool.tile([128, W], BF16, tag="s", name=f"dl{li}_{s}_{i}")
                    softplus(sb2(dl[:]), ps2(psdt[:]), bias=pps[s][:, i, 5:6])
                    g_ = spool.tile([128, W], BF16, tag="s", name=f"g{li}_{s}_{i}")
                    eng.tensor_tensor(out=g_[:], in0=dl[:], in1=u[i][:], op=ALU.mult)
                    nc.vector.memset(g_[:, 0:C0], 0.0)
                    nc.vector.memset(g_[:, 517:W], 0.0)
                    ysi = spool.tile([128, W], BF16, tag="s", name=f"ys{li}_{s}_{i}")
                    aq = [None] * 4
                    for q in range(4):
                        astf = apool.tile([128, 4, WP], BF16, tag="a", name=f"a{li}_{s}_{i}_{q}")
                        ast = astf[:, :, 0:W]
                        aq[q] = ast
                        if q == 0:
                            nc.scalar.activation(ast[:, 0, :], dl[:], AF.Exp, scale=-1.0)
                            nc.scalar.activation(ast[:, 1, :], ast[:, 0, :], AF.Square)
                            eng.tensor_tensor(out=ast[:, 2, :], in0=ast[:, 1, :], in1=ast[:, 0, :], op=ALU.mult)
                            nc.scalar.activation(ast[:, 3, :], ast[:, 1, :], AF.Square)
                        elif q == 1:
                            for j in range(3):
                                eng.tensor_tensor(out=ast[:, j, :], in0=aq[0][:, 3, :], in1=aq[0][:, j, :], op=ALU.mult)
                            nc.scalar.activation(ast[:, 3, :], aq[0][:, 3, :], AF.Square)
                        elif q == 2:
                            for j in range(4):
                                eng.tensor_tensor(out=ast[:, j, :], in0=aq[1][:, 3, :], in1=aq[0][:, j, :], op=ALU.mult)
                        else:
                            for j in range(3):
                                eng.tensor_tensor(out=ast[:, j, :], in0=aq[1][:, 3, :], in1=aq[1][:, j, :], op=ALU.mult)
                            nc.scalar.activation(ast[:, 3, :], aq[1][:, 3, :], AF.Square)
                        bstf = bpool.tile([128, 4, WP], BF16, tag="b", name=f"b{li}_{s}_{i}_{q}")
                        bst = bstf[:, :, 0:W]
                        eng.tensor_tensor(out=bst[:], in0=g_[:].unsqueeze(1).broadcast_to([128, 4, W]),
                                          in1=Bb[:, q * 4:(q + 1) * 4, :], op=ALU.mult)
                        hstf = hspool.tile([128, 4, WP], BF16, tag="h", name=f"hs{li}_{s}_{i}_{q}")
                        hst = hstf[:, :, 0:W]
                        for n in range(4):
                            nc.vector.tensor_tensor_scan(hst[:, n, :], ast[:, n, :], bst[:, n, :],
                                                         0.0, ALU.mult, ALU.add)
                        eng.tensor_tensor(out=hst[:], in0=hst[:], in1=Cb[:, q * 4:(q + 1) * 4, :], op=ALU.mult)
                        eng.tensor_tensor(out=hst[:, 0:2, :], in0=hst[:, 0:2, :], in1=hst[:, 2:4, :], op=ALU.add)
                        if q == 0:
                            eng.tensor_tensor(out=ysi[:], in0=hst[:, 0, :], in1=hst[:, 1, :], op=ALU.add)
                        else:
                            eng.tensor_tensor(out=hst[:, 0, :], in0=hst[:, 0, :], in1=hst[:, 1, :], op=ALU.add)
                            eng.tensor_tensor(out=ysi[:], in0=ysi[:], in1=hst[:, 0, :], op=ALU.add)
                    if s == 0:
                        yat = spool.tile([128, W], BF16, tag="s", name=f"ya{li}_{i}")
                        nc.vector.scalar_tensor_tensor(yat[:], u[i][:], pps[s][:, i, 6:7], ysi[:], ALU.mult, ALU.add)
                        ya.append(yat)
                    else:
                        nc.vector.scalar_tensor_tensor(ysi[:], u[i][:], pps[s][:, i, 6:7], ysi[:], ALU.mult, ALU.add)
                        eng.tensor_tensor(out=ya[i][:, 4:518], in0=ya[i][:, 4:518],
                                          in1=ysi[:, 516:2:-1], op=ALU.add)
                        eng.tensor_tensor(out=ya[i][:], in0=ya[i][:], in1=zs[i][:], op=ALU.mult)

            # ---- out_proj GEMM + residual (k-streamed, 3 m per group) ----
            for mg in range(2):
                psm = [psg.tile([128, 1024], F32, tag="g", name=f"op_ps{li}_{mg}_{m}") for m in range(3)]
                for k in range(12):
                    wt = wpool_out.tile([128, 768], BF16, tag="wo", name=f"outw{li}_{mg}_{k}")
                    nc.sync.dma_start(wt[:], outw[li, mg, k])
                    for m in range(3):
                        mm = mg * 3 + m
                        for ci, (c0, c1) in enumerate(CHK):
                            nc.tensor.matmul(psm[m][:, PCO[ci]:PCO[ci] + 262], wt[:, mm * 128:(mm + 1) * 128],
                                             ya[k][:, c0:c1], start=(k == 0), stop=(k == 11))
                for m in range(3):
                    mm = mg * 3 + m
                    nc.vector.tensor_tensor(out=sb2(h[mm][:]), in0=sb2(h[mm][:]), in1=ps2(psm[m][:]), op=ALU.add)

        if debug_h:
            for m in range(DT):
                nc.sync.dma_start(hdbg[m], h[m][:])

        # ---------------- head ----------------
        frow = rpool.tile([1, 768], F32, tag="hrow", name="frow", bufs=4)
        for m in range(DT):
            fps = pss.tile([1, 262], F32, tag="s", name=f"fps{m}")
            nc.tensor.matmul(fps[:, 0:128], h[m][:, CCLS:CCLS + 1], ident_sb[:],
                             start=True, stop=True)
            nc.scalar.copy(frow[:, m * 128:(m + 1) * 128], fps[:, 0:128])

        def ln_row(row, wcol, bcol, nm):
            mrow = rpool.tile([1, 1], F32, tag="h1", name=f"m_{nm}", bufs=8)
            nc.vector.tensor_reduce(mrow[:], row[:], mybir.AxisListType.X, ALU.add)
            nc.vector.tensor_scalar_mul(mrow[:], mrow[:], 1.0 / D)
            cen = rpool.tile([1, 768], F32, tag="hrow", name=f"c_{nm}", bufs=4)
            nc.vector.tensor_scalar_sub(cen[:], row[:], mrow[:, 0:1])
            sq_ = rpool.tile([1, 768], F32, tag="hrow", name=f"q_{nm}", bufs=4)
            nc.scalar.activation(sq_[:], cen[:], AF.Square)
            vrow = rpool.tile([1, 1], F32, tag="h1", name=f"v_{nm}", bufs=8)
            nc.vector.tensor_reduce(vrow[:], sq_[:], mybir.AxisListType.X, ALU.add)
            nc.vector.tensor_scalar_mul(vrow[:], vrow[:], 1.0 / D)
            st = rpool.tile([1, 1], F32, tag="h1", name=f"st_{nm}", bufs=8)
            nc.scalar.activation(st[:], vrow[:], AF.Ln, bias=eps_row[:, 0:1])
            rs = rpool.tile([1, 1], F32, tag="h1", name=f"rs_{nm}", bufs=8)
            nc.scalar.activation(rs[:], st[:], AF.Exp, scale=-0.5)
            nrm = rpool.tile([1, 768], F32, tag="hrow", name=f"n_{nm}", bufs=4)
            nc.vector.tensor_scalar_mul(nrm[:], cen[:], rs[:, 0:1])
            wrow = rpool.tile([1, 768], F32, tag="hrow", name=f"w_{nm}", bufs=4)
            brow = rpool.tile([1, 768], F32, tag="hrow", name=f"b_{nm}", bufs=4)
            for m in range(DT):
                fps1 = pss.tile([1, 262], F32, tag="s", name=f"w_ps_{nm}_{m}")
                nc.tensor.matmul(fps1[:, 0:128], hcon_sb[:, wcol * 6 + m:wcol * 6 + m + 1],
                                 ident_sb[:], start=True, stop=True)
                nc.scalar.copy(wrow[:, m * 128:(m + 1) * 128], fps1[:, 0:128])
                fps2 = pss.tile([1, 262], F32, tag="s", name=f"b_ps_{nm}_{m}")
                nc.tensor.matmul(fps2[:, 0:128], hcon_sb[:, bcol * 6 + m:bcol * 6 + m + 1],
                                 ident_sb[:], start=True, stop=True)
                nc.scalar.copy(brow[:, m * 128:(m + 1) * 128], fps2[:, 0:128])
            o1 = rpool.tile([1, 768], F32, tag="hrow", name=f"o_{nm}", bufs=4)
            nc.vector.tensor_tensor(out=o1[:], in0=nrm[:], in1=wrow[:], op=ALU.mult)
            nc.vector.tensor_tensor(out=o1[:], in0=o1[:], in1=brow[:], op=ALU.add)
            return o1

        f1 = ln_row(frow, 0, 1, "ln1")
        f2 = ln_row(f1, 2, 3, "ln2")
        f2b = rpool.tile([1, 768], BF16, tag="hrow", name="f2b", bufs=4)
        nc.vector.tensor_copy(f2b[:], f2[:])
        fb2 = rpool.tile([NCLS, 768], BF16, tag="hbc", name="fbc", bufs=2)
        fdr = dpool.tile([1, 768], BF16, tag="fdr", name="fdr")
        nc.sync.dma_start(fdr[:], f2b[:])
        nc.sync.dma_start(fb2[:], fdr[:].broadcast_to([NCLS, 768]))
        pr = rpool.tile([NCLS, 768], BF16, tag="hbc", name="pr", bufs=2)
        nc.vector.tensor_tensor(out=pr[:], in0=fb2[:], in1=hw_sb[:], op=ALU.mult)
        lg = rpool.tile([NCLS, 1], F32, tag="h1", name="lg", bufs=8)
        nc.vector.tensor_reduce(lg[:], pr[:], mybir.AxisListType.X, ALU.add)
        nc.vector.tensor_tensor(out=lg[:], in0=lg[:], in1=hb_sb[:], op=ALU.add)
        nc.sync.dma_start(outp[:].unsqueeze(-1), lg[:])
    return nc


# ---------------- host side ----------------
_cache = {}


def _prep(inputs, depth=DEPTH):
    f32 = np.float32
    bf = ml_dtypes.bfloat16
    g = lambda k: np.asarray(inputs[k], f32)
    assert np.allclose(np.exp(g('A_log_f')), np.arange(1, N + 1, dtype=f32)[None, None, :], atol=1e-4), "A structure"
    assert np.allclose(np.exp(g('A_log_b')), np.arange(1, N + 1, dtype=f32)[None, None, :], atol=1e-4), "A structure"

    w = {}
    pwf = g('patch_w').reshape(D, 256)
    lhsT = pwf.T.astype(bf)
    w['pw_p'] = np.stack([lhsT[0:128], lhsT[128:256]])
    pbcv = np.zeros((128, 12), f32)
    for m in range(DT):
        pbcv[:, m] = g('patch_b')[m * 128:(m + 1) * 128]
        pbcv[:, 6 + m] = g('cls_token').reshape(D)[m * 128:(m + 1) * 128]
    w['pbc_p'] = pbcv

    inw_l, inb_l, outw_l, xw_l, dtw_l, pp_l = [], [], [], [], [], []
    for i in range(depth):
        Wf = (g('in_proj_w')[i] * g('norm_w')[i][None, :]).T.astype(bf)
        inw_l.append(np.stack([Wf[k * 128:(k + 1) * 128] for k in range(6)]))
        ib = (g('in_proj_w')[i] @ g('norm_b')[i]).astype(f32)
        inb_l.append(ib.reshape(24, 128).T.copy())
        oT = g('out_proj_w')[i].T.astype(bf)   # [1536, 768]
        ow = np.stack([oT[k * 128:(k + 1) * 128] for k in range(12)])
        outw_l.append(np.stack([ow, ow]))      # same for both m-groups
        xw_s, dtw_s, pp_s = [], [], []
        for s, sfx in enumerate(('f', 'b')):
            xpr = g('x_proj_w_' + sfx)[i]
            xe = np.zeros((112, E), f32)
            xe[0:48] = xpr[0:48]; xe[64:80] = xpr[48:64]; xe[96:112] = xpr[64:80]
            xeT = xe.T.astype(bf)
            xw_s.append(np.stack([xeT[k * 128:(k + 1) * 128] for k in range(12)]))
            dtw_s.append(g('dt_proj_w_' + sfx)[i].T.astype(bf))
            ppv = np.zeros((128, ET, 8), f32)
            for it in range(ET):
                sl = slice(it * 128, (it + 1) * 128)
                ppv[:, it, 0:4] = g('conv_w_' + sfx)[i][sl]
                ppv[:, it, 4] = g('conv_b_' + sfx)[i][sl]
                ppv[:, it, 5] = g('dt_proj_b_' + sfx)[i][sl]
                ppv[:, it, 6] = g('D_' + sfx)[i][sl]
            pp_s.append(ppv)
        xw_l.append(np.stack(xw_s)); dtw_l.append(np.stack(dtw_s)); pp_l.append(np.stack(pp_s))
    w['inw_p'] = np.stack(inw_l).astype(bf); w['inb_p'] = np.stack(inb_l)
    w['outw_p'] = np.stack(outw_l).astype(bf); w['xw_p'] = np.stack(xw_l).astype(bf)
    w['dtw_p'] = np.stack(dtw_l).astype(bf); w['pp_p'] = np.stack(pp_l)
    if depth < DEPTH:
        for k in ('inw_p', 'inb_p', 'outw_p', 'xw_p', 'dtw_p', 'pp_p'):
            pad = [(0, DEPTH - depth)] + [(0, 0)] * (w[k].ndim - 1)
            w[k] = np.pad(w[k], pad)

    hc = np.zeros((128, 24), f32)
    for m in range(DT):
        sl = slice(m * 128, (m + 1) * 128)
        hc[:, m] = g('fnorm_w')[sl]; hc[:, 6 + m] = g('fnorm_b')[sl]
        hc[:, 12 + m] = g('head_norm_w')[sl]; hc[:, 18 + m] = g('head_norm_b')[sl]
    w['hcon_p'] = hc
    w['hwr_p'] = g('head_w').astype(bf)
    w['hbr_p'] = g('head_b').reshape(NCLS, 1)
    w['ident_p'] = np.eye(128, dtype=f32)
    return w


def _prep_x(x):
    xs = []
    for c in range(x.shape[0]):
        xc = np.asarray(x[c, 0], np.float32)
        p = xc.reshape(8, 16, 64, 16).transpose(1, 3, 0, 2).reshape(256, 512)
        xs.append(np.stack([p[0:128], p[128:256]]).copy())
    return xs


def kernel(**inputs):
    if 'nc' not in _cache:
        nc = bacc.Bacc("TRN2", target_bir_lowering=False, debug=False, num_devices=B)
        build(nc, DEPTH, sim_mode=False)
        nc.compile()
        _cache['nc'] = nc
    nc = _cache['nc']
    w = _prep(inputs, DEPTH)
    xs = _prep_x(np.asarray(inputs['x']))
    in_maps = []
    for c in range(B):
        m = dict(w)
        m['x_p'] = xs[c]
        in_maps.append(m)
    res = run_bass_kernel_spmd(nc, in_maps, core_ids=list(range(B)))
    return np.stack([np.asarray(res.results[c]['out'], np.float32) for c in range(B)])
